# revision 2
# baseline (speedup 1.0000x reference)
"""Trainium2 Bass kernel for nn_Attention (B=4, S=2048, D=1024, H=16, hd=64).

v2: fp8(e4m3) DoubleRow matmuls for the Q/K projections and attnV (2x PE
stream rate; LDWEIGHTS hides under the 512-cycle stream), bf16 for
scores / V-projection / out-projection, and softmax exp split between
ScalarE (exact LUT) and a custom DVE op (deg-2 poly + 4 squarings).

Sharding (zero-communication, as baseline): core c handles batch c//2 and
query-half c%2; computes K,V for the whole sequence, Q for its 1024
queries, all 16 heads, and the output projection for its rows.

Scaling: Wq/Wk stored x256 in fp8 (subnormal protection), folded back at
evac; Q additionally pre-scaled 1/128 so PSUM scores hold z = s/128 and
exp computes EPS*e^(16z) (EPS=8 keeps eP < 240, the TRN e4m3 max).
The V ones-column gives the softmax denominator; normalization cancels
EPS. HAM note: filler matmuls are spread across every attention block to
keep the PE clock at 8/8.
"""

import math
import sys

import numpy as np

B, S, D, H, HD = 4, 2048, 1024, 16, 64
QH = 1024
NC_ = 8
WS = 256.0          # Wq/Wk fp8 scale
EPS_SCALE = 8.0     # eP = EPS_SCALE * e^(s/8)
LNEPS = math.log(EPS_SCALE)
ZFIT = 0.24
# odd kts -> DVE except kt=1 (block-start group runs both on SE while
# the DVE drains the previous block's evac chain)
DVE_KT = frozenset((3, 5, 7, 9, 11, 13, 15))

_cache = {}


def _fit_poly():
    g = np.linspace(-ZFIT, ZFIT, 4001)
    t = (EPS_SCALE ** (1.0 / 16.0)) * np.exp(g)
    A = np.stack([np.ones_like(g), g, g * g], 1)
    w = 1.0 / t
    c = np.linalg.lstsq(A * w[:, None], t * w, rcond=None)[0]
    return [float(v) for v in c]


C0F, C1F, C2F = _fit_poly()


def _register_exp16():
    """Register the custom DVE op computing (C0 + z(C1 + z C2))^16."""
    import concourse.dve_ops as dops
    from concourse.dve_spec import Spec, Src0, C0, C1, C2, sq, lower
    from concourse.dve_uop import DveOpSpec
    from concourse.bass_utils import dve_ver_for

    if "EXP16_ANT" in dops._SUB_OPCODE_FOR_NAME:
        return next(op for op in dops.OPS if op.name == "EXP16_ANT")
    z = Src0
    body = sq(sq(sq(sq(C0 + z * (C1 + z * C2)))))

    def _ref_exp16(in0, in1, c0, c1, c2):
        xx = in0.astype(np.float32)
        return (c0 + xx * (c1 + xx * c2)) ** 16

    spec = Spec(body=body, reference=_ref_exp16)
    row = max(dops._SUB_OPCODE_FOR_NAME.values()) + 1
    assert row < 0x20
    dops._SUB_OPCODE_FOR_NAME["EXP16_ANT"] = row
    ver = dve_ver_for("TRN2")
    sha = DveOpSpec(
        name="EXP16_ANT", opcode=row, uops=lower(spec, ver=ver), rd1_en=False
    ).sha(ver)
    op = dops.DveOp("EXP16_ANT", spec, subdim=False, uops_sha={ver: sha})
    dops.OPS.append(op)
    dops.CUSTOM_DVE_SPECS["EXP16_ANT"] = spec
    return op


def _build_nc():
    sys.path.insert(0, "/opt/trn_rl_repo")
    import concourse.bass as bass
    from concourse import bacc
    import concourse.mybir as mybir
    import concourse.tile as tile
    from contextlib import ExitStack

    EXP16 = _register_exp16()

    F32 = mybir.dt.float32
    BF16 = mybir.dt.bfloat16
    F8 = mybir.dt.float8e4
    MULT = mybir.AluOpType.mult
    ADD = mybir.AluOpType.add
    Exp = mybir.ActivationFunctionType.Exp
    DR = mybir.MatmulPerfMode.DoubleRow

    nc = bacc.Bacc()
    x_d = nc.declare_dram_parameter("xb", [128, 8, S], F8, isOutput=False)
    xB_d = nc.declare_dram_parameter("xbb", [128, 8, S], BF16, isOutput=False)
    wq_d = nc.declare_dram_parameter("wq", [128, 8, D], F8, isOutput=False)
    wk_d = nc.declare_dram_parameter("wk", [128, 8, D], F8, isOutput=False)
    wv_d = nc.declare_dram_parameter("wv", [128, 8, D], BF16, isOutput=False)
    wp_d = nc.declare_dram_parameter("wp", [128, 8, D], BF16, isOutput=False)
    bqp_d = nc.declare_dram_parameter("bqp", [128, 8], F32, isOutput=False)
    bkp_d = nc.declare_dram_parameter("bkp", [128, 8], F32, isOutput=False)
    bvr_d = nc.declare_dram_parameter("bvr", [1, D], BF16, isOutput=False)
    bpr_d = nc.declare_dram_parameter("bpr", [1, D], BF16, isOutput=False)
    out_d = nc.declare_dram_parameter("out", [QH, D], F32, isOutput=True)
    DBG = bool(__import__("os").environ.get("KDBG"))
    if DBG:
        kt_d = nc.declare_dram_parameter("ktd", [128, 8 * S], F32, isOutput=True)
        qt_d = nc.declare_dram_parameter("qtd", [128, 8 * QH], F32, isOutput=True)
        va_d = nc.declare_dram_parameter("vad", [128, 8 * 2 * 16 * 65], F32, isOutput=True)
        ot_d = nc.declare_dram_parameter("otd", [128, 8 * QH], F32, isOutput=True)

    with ExitStack() as ctx:
        tc = ctx.enter_context(tile.TileContext(nc))

        const = ctx.enter_context(tc.tile_pool(name="const", bufs=1))
        ones1 = const.tile([1, 128], BF16)
        nc.vector.memset(ones1[:, :], 1.0)
        ones_lo = const.tile([1, 128], BF16)
        nc.vector.memset(ones_lo[:, 0:64], 1.0)
        nc.vector.memset(ones_lo[:, 64:128], 0.0)
        ones_hi = const.tile([1, 128], BF16)
        nc.vector.memset(ones_hi[:, 0:64], 0.0)
        nc.vector.memset(ones_hi[:, 64:128], 1.0)
        lneps = const.tile([128, 1], F32)
        nc.vector.memset(lneps[:, :], LNEPS)
        bqp = const.tile([128, 8], F32)
        nc.sync.dma_start(out=bqp[:, :], in_=bqp_d[:, :])
        bkp = const.tile([128, 8], F32)
        nc.sync.dma_start(out=bkp[:, :], in_=bkp_d[:, :])
        bvr = const.tile([1, D], BF16)
        nc.sync.dma_start(out=bvr[:, :], in_=bvr_d[:, :])
        bpr = const.tile([1, D], BF16)
        nc.sync.dma_start(out=bpr[:, :], in_=bpr_d[:, :])

        big = ctx.enter_context(tc.tile_pool(name="big", bufs=1))
        KT = big.tile([128, 8 * S], BF16)          # [p(2 heads hd), (j, s)]
        QT = big.tile([128, 8 * QH], BF16)         # [p(2 heads hd), (j, q)]
        Vaug = big.tile([128, 8 * 2 * 16 * 65], F8)  # [p, (g, t, h, 65)]
        oT = big.tile([128, 8 * QH], BF16)         # [p(2 heads d), (j, q)]

        KTv = KT[:, :].rearrange("p (j s) -> p j s", j=8)
        QTv = QT[:, :].rearrange("p (j q) -> p j q", j=8)
        Vv = Vaug[:, :].rearrange("p (g t h e) -> p g t h e", g=8, t=2, h=16)
        oTv = oT[:, :].rearrange("p (j q) -> p j q", j=8)

        # ones-columns of Vaug give the softmax denominator for free
        Vflat = Vaug[:, :].rearrange("p (a e) -> p a e", e=65)
        nc.vector.memset(Vflat[:, :, 64:65], 1.0)

        apool = ctx.enter_context(tc.tile_pool(name="att", bufs=4))
        npool = ctx.enter_context(tc.tile_pool(name="attn", bufs=2))
        xTp_cm = tc.tile_pool(name="xTp", bufs=1)
        xTp = xTp_cm.__enter__()
        xT = xTp.tile([128, 8 * S], F8)            # [p, (dt, s)] fp8 for kq
        xTv = xT[:, :].rearrange("p (d s) -> p d s", d=8)

        # PSUM: ps ring 2x[128,1024] (4 banks) + pk ring 2x[128,512]
        # (2 banks) + po ring 2x[65,512] (2 banks) = 8 banks
        psm = ctx.enter_context(tc.tile_pool(name="psm", bufs=2, space="PSUM"))
        pso = ctx.enter_context(tc.tile_pool(name="pso", bufs=2, space="PSUM"))

        # bias rows broadcast to all 128 partitions once (bf16 V/proj paths)
        bvb = const.tile([128, D], BF16)
        bpb = const.tile([128, D], BF16)
        pbias = psm.tile([128, 1024], F32, tag="ps", name="pbias")
        for nh in range(2):
            nc.tensor.matmul(pbias[:, nh * 512:(nh + 1) * 512], ones1[:, :],
                             bvr[:, nh * 512:(nh + 1) * 512],
                             start=True, stop=True)
        nc.vector.tensor_copy(bvb[:, :], pbias[:, :])
        pbias2 = psm.tile([128, 1024], F32, tag="ps", name="pbias2")
        for nh in range(2):
            nc.tensor.matmul(pbias2[:, nh * 512:(nh + 1) * 512], ones1[:, :],
                             bpr[:, nh * 512:(nh + 1) * 512],
                             start=True, stop=True)
        nc.vector.tensor_copy(bpb[:, :], pbias2[:, :])

        wkq_cm = tc.tile_pool(name="wkq", bufs=1)
        wkq = wkq_cm.__enter__()
        wkt = wkq.tile([128, 8 * D], F8)
        wqt = wkq.tile([128, 8 * D], F8)
        wkv_ = wkt[:, :].rearrange("p (d n) -> p d n", d=8)
        wqv_ = wqt[:, :].rearrange("p (d n) -> p d n", d=8)

        def kq_chunks(j):
            steps = []
            ksteps = []
            for sc in range(4):
                def mk_k(sc=sc):
                    pkc = pso.tile([128, 512], F32, tag="pk",
                                   name=f"pk{j}_{sc}")
                    for t in range(4):
                        nc.tensor.matmul(
                            pkc[:, :],
                            wkv_[:, 2 * t:2 * t + 2, j * 128:(j + 1) * 128],
                            xTv[:, 2 * t:2 * t + 2, sc * 512:(sc + 1) * 512],
                            start=(t == 0), stop=(t == 3), perf_mode=DR,
                        )
                    nc.vector.tensor_scalar(
                        KTv[:, j, sc * 512:(sc + 1) * 512], pkc[:, :],
                        bkp[:, j:j + 1], 1.0 / WS, op0=ADD, op1=MULT)
                ksteps.append(mk_k)
            for qc in range(2):
                def mk_q(qc=qc):
                    pqc = pso.tile([128, 512], F32, tag="pk",
                                   name=f"pq{j}_{qc}")
                    for t in range(4):
                        nc.tensor.matmul(
                            pqc[:, :],
                            wqv_[:, 2 * t:2 * t + 2, j * 128:(j + 1) * 128],
                            xTv[:, 2 * t:2 * t + 2, qc * 512:(qc + 1) * 512],
                            start=(t == 0), stop=(t == 3), perf_mode=DR,
                        )
                    nc.vector.tensor_scalar(
                        QTv[:, j, qc * 512:(qc + 1) * 512], pqc[:, :],
                        bqp[:, j:j + 1], 1.0 / (WS * 128.0),
                        op0=ADD, op1=MULT)
                steps.append(mk_q)
            return [ksteps[0]] + steps + ksteps[1:]

        # ---- Phase A: loads, first-needed-first --------------------------
        nc.sync.dma_start(out=wkv_[:, :, 0:128], in_=wk_d[:, :, 0:128])
        for dt_ in range(8):
            nc.sync.dma_start(out=xTv[:, dt_, 0:512],
                              in_=x_d[:, dt_, 0:512])
        nc.sync.dma_start(out=wqv_[:, :, 0:128], in_=wq_d[:, :, 0:128])
        for dt_ in range(8):
            nc.sync.dma_start(out=xTv[:, dt_, 512:1024],
                              in_=x_d[:, dt_, 512:1024])
        # bf16 x copy + wv for the V projection (accuracy-critical path)
        xBp_cm = tc.tile_pool(name="xBp", bufs=1)
        xBp = xBp_cm.__enter__()
        xB = xBp.tile([128, 8 * S], BF16)
        xBv = xB[:, :].rearrange("p (d s) -> p d s", d=8)
        wvp_cm = tc.tile_pool(name="wv", bufs=1)
        wvp = wvp_cm.__enter__()
        wvt = wvp.tile([128, 8 * D], BF16)
        wvv_ = wvt[:, :].rearrange("p (d n) -> p d n", d=8)
        for sc in range(2, 4):
            for dt_ in range(8):
                nc.sync.dma_start(
                    out=xTv[:, dt_, sc * 512:(sc + 1) * 512],
                    in_=x_d[:, dt_, sc * 512:(sc + 1) * 512])
        nc.sync.dma_start(out=wvv_[:, :, :], in_=wv_d[:, :, :])
        for sc in range(4):
            for dt_ in range(8):
                nc.sync.dma_start(
                    out=xBv[:, dt_, sc * 512:(sc + 1) * 512],
                    in_=xB_d[:, dt_, sc * 512:(sc + 1) * 512])
        kq0 = kq_chunks(0)
        for step in kq0:
            step()

        def attn_scores_exp(j, qc, g):
            qsl = slice(qc * 512, (qc + 1) * 512)
            kts = (2 * g, 2 * g + 1)
            eP = apool.tile([128, 2 * 1024], F8, tag="eP")
            ePv = eP[:, :].rearrange("p (t q) -> p t q", t=2)
            pss = []
            for kt in kts:
                ps = psm.tile([128, 1024], F32, tag="ps",
                              name=f"ps{j}_{qc}_{kt}")
                pss.append(ps)
                nc.tensor.matmul(
                    ps[:, 0:512],
                    KTv[0:64, j, kt * 128:(kt + 1) * 128],
                    QTv[0:64, j, qsl],
                    start=True, stop=True, tile_position=(0, 0))
                nc.tensor.matmul(
                    ps[:, 512:1024],
                    KTv[64:128, j, kt * 128:(kt + 1) * 128],
                    QTv[64:128, j, qsl],
                    start=True, stop=True, tile_position=(64, 0))
            for i, (kt, ps) in enumerate(zip(kts, pss)):
                if kt not in DVE_KT:
                    nc.scalar.activation(ePv[:, i, :], ps[:, :], Exp,
                                         bias=lneps[:, 0:1], scale=16.0)
                else:
                    nc.vector._custom_dve(EXP16, out=ePv[:, i, :],
                                          in0=ps[:, :], s0=C0F, s1=C1F,
                                          imm2=C2F)
            return ePv

        def attn_v(j, qc, g, ePv, poA, poB):
            for h in range(2):
                nc.tensor.matmul(
                    (poA if h == 0 else poB)[:, :],
                    Vv[:, g, :, 2 * j + h, 0:65],
                    ePv[:, :, h * 512:(h + 1) * 512],
                    start=(g == 0), stop=(g == 7), perf_mode=DR,
                )

        def attn_group(j, qc, g, poA, poB):
            ePv = attn_scores_exp(j, qc, g)
            attn_v(j, qc, g, ePv, poA, poB)

        rps = {}

        def attn_evac(j, qc, poA, poB):
            # l-rows must reach partition 0 via plain tensor_copy first: a
            # custom-DVE read at a nonzero PSUM partition offset misreads
            # on HW (returns row 0)
            lp = npool.tile([1, 1024], F32, tag="lp", name=f"lp{j}_{qc}")
            nc.vector.tensor_copy(lp[0:1, 0:512], poA[64:65, :])
            nc.vector.tensor_copy(lp[0:1, 512:1024], poB[64:65, :])
            rp = npool.tile([1, 1024], F32, tag="rp", name=f"rp{j}_{qc}")
            nc.vector.reciprocal_approx_fast(rp[0:1, :], lp[0:1, :])
            rpb = npool.tile([1, 1024], BF16, tag="rpb", name=f"rpb{j}_{qc}")
            nc.scalar.copy(rpb[:, :], rp[:, :])
            stage = npool.tile([128, 512], BF16, tag="stg",
                               name=f"stg{j}_{qc}")
            nc.scalar.copy(stage[0:64, :], poA[0:64, :])
            nc.scalar.copy(stage[64:128, :], poB[0:64, :])
            rps[(j, qc)] = (rpb, stage)

        def attn_norm_tail(j, qc):
            qsl = slice(qc * 512, (qc + 1) * 512)
            rpb, stage = rps.pop((j, qc))
            pbc = pso.tile([128, 512], F32, tag="pk", name=f"pbc{j}_{qc}")
            nc.tensor.matmul(pbc[:, :], ones_lo[0:1, :], rpb[0:1, 0:512],
                             start=True, stop=False)
            nc.tensor.matmul(pbc[:, :], ones_hi[0:1, :], rpb[0:1, 512:1024],
                             start=False, stop=True)
            rbc = npool.tile([128, 512], F32, tag="rbc", name=f"rbc{j}_{qc}")
            nc.vector.tensor_copy(rbc[:, :], pbc[:, :])
            nc.gpsimd.tensor_tensor(oTv[:, j, qsl], stage[:, :], rbc[:, :],
                                    MULT)

        pending_norm = []

        def flush_norm():
            while pending_norm:
                pending_norm.pop(0)()

        def attn_block(j, qc, interleave=None):
            poA = pso.tile([65, 512], F32, tag="po", name=f"poA{j}_{qc}")
            poB = pso.tile([65, 512], F32, tag="po", name=f"poB{j}_{qc}")
            nsteps = len(interleave) if interleave else 0
            si = 0
            # software pipeline: attnV for group g-1 is emitted after the
            # scores+exp of group g, so the PE never waits on an exp result
            pend = None
            for g in range(8):
                ePv = attn_scores_exp(j, qc, g)
                if pend is not None:
                    attn_v(j, qc, pend[0], pend[1], poA, poB)
                pend = (g, ePv)
                if interleave and si < nsteps and g in (1, 3, 5):
                    interleave[si]()
                    si += 1
                if g == 6:
                    flush_norm()
            attn_v(j, qc, pend[0], pend[1], poA, poB)
            while interleave and si < nsteps:
                interleave[si]()
                si += 1
            attn_evac(j, qc, poA, poB)
            pending_norm.append(lambda j=j, qc=qc: attn_norm_tail(j, qc))

        # ---- V projection (bf16), pipelined with block (0,0) -------------
        nc.sync.dma_start(out=wkv_[:, :, 128:D], in_=wk_d[:, :, 128:D])
        nc.sync.dma_start(out=wqv_[:, :, 128:D], in_=wq_d[:, :, 128:D])

        def v_st(st):
            pv = psm.tile([128, 1024], F32, tag="ps", name=f"pv{st}")
            for dt_ in range(8):
                for nh in range(2):
                    nc.tensor.matmul(
                        pv[:, nh * 512:(nh + 1) * 512],
                        xBv[:, dt_, st * 128:(st + 1) * 128],
                        wvv_[:, dt_, nh * 512:(nh + 1) * 512],
                        start=(dt_ == 0), stop=(dt_ == 7),
                    )
            dst = Vv[:, st // 2, st % 2, :, 0:64]
            src_ = pv[:, :].rearrange("p (h d) -> p h d", h=16)
            bsr = bvb[:, :].rearrange("p (h d) -> p h d", h=16)
            nc.vector.tensor_tensor(dst, src_, bsr, ADD)

        poA0 = pso.tile([65, 512], F32, tag="po", name="poA0_0")
        poB0 = pso.tile([65, 512], F32, tag="po", name="poB0_0")
        for g in range(8):
            v_st(2 * g)
            v_st(2 * g + 1)
            attn_group(0, 0, g, poA0, poB0)
        attn_evac(0, 0, poA0, poB0)
        pending_norm.append(lambda: attn_norm_tail(0, 0))
        wvp_cm.__exit__(None, None, None)
        xBp_cm.__exit__(None, None, None)

        filler = []
        for m in range(1, 8):
            filler += kq_chunks(m)
        blocks = [(0, 1)] + [(jj, qq) for jj in range(1, 8) for qq in (0, 1)]
        fi = 0
        for bi, (jj, qq) in enumerate(blocks[:-1]):
            take = filler[fi:fi + 3]
            fi += 3
            attn_block(jj, qq, interleave=take)
        while fi < len(filler):
            filler[fi]()
            fi += 1
        flush_norm()
        wkq_cm.__exit__(None, None, None)
        xTp_cm.__exit__(None, None, None)

        # ---- Phase D: output projection (bf16) + final block (7,1) -------
        with tc.tile_pool(name="wp", bufs=1) as wpp, \
             tc.tile_pool(name="ystg", bufs=2) as ypool:
            wpt = wpp.tile([128, 8 * D], BF16)
            wpv_ = wpt[:, :].rearrange("p (d n) -> p d n", d=8)
            nc.sync.dma_start(out=wpv_[:, :, :], in_=wp_d[:, :, :])

            def proj_nh(qt, nh):
                ph = pso.tile([128, 512], F32, tag="pk", name=f"ph{qt}_{nh}")
                for jj in range(8):
                    nc.tensor.matmul(
                        ph[:, :],
                        oTv[:, jj, qt * 128:(qt + 1) * 128],
                        wpv_[:, jj, nh * 512:(nh + 1) * 512],
                        start=(jj == 0), stop=(jj == 7),
                    )
                ys = ypool.tile([128, 512], F32, tag="ysh")
                nc.vector.tensor_tensor(
                    ys[:, :], ph[:, :], bpb[:, nh * 512:(nh + 1) * 512], ADD)
                nc.sync.dma_start(
                    out=out_d[qt * 128:(qt + 1) * 128,
                              nh * 512:(nh + 1) * 512], in_=ys[:, :])

            if DBG:
                with tc.tile_pool(name="dbg", bufs=1) as dbgp:
                    for (dd, tt, w_) in ((kt_d, KT, 8 * S), (qt_d, QT, 8 * QH),
                                         (va_d, Vaug, 8 * 2 * 16 * 65),
                                         (ot_d, oT, 8 * QH)):
                        for c0 in range(0, w_, 4096):
                            cw = min(4096, w_ - c0)
                            db = dbgp.tile([128, cw], F32, tag="db")
                            nc.vector.tensor_copy(db[:, :], tt[:, c0:c0 + cw])
                            nc.sync.dma_start(out=dd[:, c0:c0 + cw], in_=db[:, :])
            halves = [(qt, nh) for qt in range(8) for nh in range(2)]
            attn_block(7, 1,
                       interleave=[lambda qt=qt, nh=nh: proj_nh(qt, nh)
                                   for qt, nh in halves[0:3]])
            flush_norm()
            for qt, nh in halves[3:]:
                proj_nh(qt, nh)

    nc.finalize()
    return nc


def _in_maps(x, W_qkv, b_qkv, W_proj, b_proj):
    import ml_dtypes
    f8 = ml_dtypes.float8_e4m3
    bf16 = ml_dtypes.bfloat16
    x = np.asarray(x, np.float32)
    W_qkv = np.asarray(W_qkv, np.float32)
    b_qkv = np.asarray(b_qkv, np.float32)
    W_proj = np.asarray(W_proj, np.float32)
    b_proj = np.asarray(b_proj, np.float32)

    def pack(w, dt, scale=1.0):  # [D, D] -> [128, 8, D]
        return np.ascontiguousarray(
            (w * scale).reshape(8, 128, D).transpose(1, 0, 2).astype(dt))

    Wq8 = pack(W_qkv[:, 0:D], f8, WS)
    Wk8 = pack(W_qkv[:, D:2 * D], f8, WS)
    Wvb = pack(W_qkv[:, 2 * D:3 * D], bf16)
    Wpb = pack(W_proj, bf16)
    bq, bk, bv = b_qkv[0:D], b_qkv[D:2 * D], b_qkv[2 * D:3 * D]
    bqp = np.ascontiguousarray((WS * bq).reshape(8, 128).T)
    bkp = np.ascontiguousarray((WS * bk).reshape(8, 128).T)
    bvr = np.ascontiguousarray(bv.reshape(1, D).astype(bf16))
    bpr = np.ascontiguousarray(b_proj.reshape(1, D).astype(bf16))
    maps = []
    for c in range(NC_):
        b, qh = c // 2, c % 2
        xb = np.concatenate(
            [x[b, qh * QH:(qh + 1) * QH], x[b, (1 - qh) * QH:(2 - qh) * QH]],
            axis=0)  # [S, D] own queries first
        xt = xb.T.reshape(8, 128, S).transpose(1, 0, 2)
        maps.append({
            "xb": np.ascontiguousarray(xt.astype(f8)),
            "xbb": np.ascontiguousarray(xt.astype(bf16)),
            "wq": Wq8, "wk": Wk8, "wv": Wvb, "wp": Wpb,
            "bqp": bqp, "bkp": bkp, "bvr": bvr, "bpr": bpr,
        })
    return maps


def run(x, W_qkv, b_qkv, W_proj, b_proj, trace=False, tmpdir=None):
    sys.path.insert(0, "/opt/trn_rl_repo")
    from concourse.bass_utils import run_bass_kernel_spmd

    if "nc" not in _cache:
        _cache["nc"] = _build_nc()
    nc = _cache["nc"]
    maps = _in_maps(x, W_qkv, b_qkv, W_proj, b_proj)
    res = run_bass_kernel_spmd(nc, maps, core_ids=list(range(NC_)),
                               trace=trace, tmpdir=tmpdir)
    y = np.empty((B, S, D), np.float32)
    for c in range(NC_):
        b, qh = c // 2, c % 2
        y[b, qh * QH:(qh + 1) * QH] = res.results[c]["out"]
    return y, res


def kernel(x, W_qkv, b_qkv, W_proj, b_proj):
    y, _ = run(x, W_qkv, b_qkv, W_proj, b_proj, trace=False)
    return y
